# revision 16
# baseline (speedup 1.0000x reference)
"""DecoderLSTM Trainium2 kernel v12 (8 NeuronCores, SPMD, no collectives).

Split chosen for the "memory" target regime:
  - The LSTM scan is 0.03 GFLOP of latency-bound serial math; it runs on
    the HOST in float32 numpy, exactly mirroring the reference semantics.
  - The DEVICE does the memory/compute-dominant work: the [4096, 50257]
    logits projection, vocab-sharded 8 ways (6283 vocab rows per core,
    12 chunks x 484 + 1 x 475). Token-stationary GEMM: a [512 hdim x
    128 token] block is the PE stationary operand, Wf^T streams as the
    moving operand. The MM stream runs at the bf16 PE wall (~343 us).

v12 changes vs v11 (360.5 us) / v8 (385.2 us):
  - all inputs/outputs move in large blocked transfers (0.25 - 3 MB,
    2 - 23 KB partition lines): single DMAs split across all 16 SDMA
    engines only reach full rate at ~1 MB (~1 KB-line 124 KB transfers
    cap aggregate DMA at ~190 GB/s).
  - vocab chunks in blocks [1, 6, 4, 2]: the first 0.5 MB weight block
    unblocks a 26 us sweep over all 32 token tiles, hiding the rest of
    the input load.
  - per-core vocab shard is the exact 6283 = ceil(50257/8); last chunk
    475 wide.
  - last two token tiles of the final block store per-chunk so the
    late-drain tail is one 0.12 MB store instead of 0.24 MB+.
  - sweep-0 results batched 8 token tiles per store (4 x 0.97 MB).
  - 9 dummy matmuls on a zeroed tile warm the PE clock (HAM 1.2 ->
    2.4 GHz takes ~3.4 us of activity) while the first inputs land.
"""

import os
import numpy as np
import ml_dtypes

V, E, H, B, S = 50257, 512, 512, 32, 128
NC_ = 8
W = 484                         # vocab chunk slot width (<=512 PSUM bank)
NCH = 13                        # chunks per core
VSH = 6283                      # per-core vocab shard = ceil(V / 8)
VPAD = VSH * NC_                # 50264
NTOK = B * S                    # 4096 tokens, token = t*32 + b
NG = NTOK // 128                # 32 token tiles of 128

# chunk widths: 12 x 484 + 475 = 6283
CW = [W] * (NCH - 1) + [VSH - W * (NCH - 1)]
CBLK = [(0, 1), (1, 6), (7, 4), (11, 2)]          # (chunk0, nchunks)
HBLK = [(0, 1), (1, 1), (2, 2), (4, 2), (6, 2), (8, 8), (16, 8), (24, 8)]
GB0 = 8                         # sweep-0 token tiles per batched store


def _bw(b):
    """total output width of chunk block b"""
    c0, n = CBLK[b]
    return sum(CW[c0:c0 + n])


_cache = {}


def _build_program():
    import concourse.bass as bass
    import concourse.bacc as bacc
    import concourse.tile as tile
    from concourse import mybir

    bf16 = mybir.dt.bfloat16
    f32 = mybir.dt.float32

    nc = bacc.Bacc("TRN2", target_bir_lowering=False, debug=False,
                   enable_asserts=False, num_devices=NC_)

    # hT block i: [128, n*512], line h: [g' * 512 + 128k + j]
    d_ht = [nc.dram_tensor(f"hTb{i}", [128, n * 512], bf16,
                           kind="ExternalInput").ap()
            for i, (g0, n) in enumerate(HBLK)]
    # wf block b: [128, n*4*484] (484-wide slots, last chunk padded),
    # line p: [ci * 4*484 + k * 484 + w]
    d_wf = [nc.dram_tensor(f"wfb{b}", [128, n * 4 * W], bf16,
                           kind="ExternalInput").ap()
            for b, (c0, n) in enumerate(CBLK)]
    # out block 0: [NG/GB0, 128, GB0*484] (g batched for big stores);
    # out block b>0: [NG, 128, bw] with bw = sum of chunk widths
    d_ob = [nc.dram_tensor("ob0", [NG // GB0, 128, GB0 * W], bf16,
                           kind="ExternalOutput").ap()]
    d_ob += [nc.dram_tensor(f"ob{b}", [NG, 128, _bw(b)], bf16,
                            kind="ExternalOutput").ap()
             for b in range(1, len(CBLK))]

    with tile.TileContext(nc) as tc:
        wp = tc.alloc_tile_pool(name="wf", bufs=1)
        hp = tc.alloc_tile_pool(name="ht", bufs=1)
        s0 = tc.alloc_tile_pool(name="st0", bufs=1)
        sp = [None,
              tc.alloc_tile_pool(name="stp1", bufs=3),
              tc.alloc_tile_pool(name="stp2", bufs=4),
              tc.alloc_tile_pool(name="stp3", bufs=4)]
        wu = tc.alloc_tile_pool(name="wu", bufs=1)
        pp = tc.alloc_tile_pool(name="ps", bufs=8, space="PSUM")

        # HAM warmup: dummy matmuls on a zeroed tile keep the PE busy
        # (starting its 3.4 us HAM activity window early) exactly until
        # the first input DMA lands, so real MMs start ASAP and reach
        # 2.4 GHz with minimal cold time.
        wt = wu.tile([128, 256], bf16, tag="warm")
        nc.vector.memset(wt[:], 0.0)
        wps = pp.tile([128, 512], f32, tag="pj")
        for _ in range(10):
            nc.tensor.matmul(wps[:, 0:256], wt[:, 0:128], wt[:],
                             start=True, stop=True)

        # input DMAs, first-needed-first; each is one large multi-engine
        # transfer (0.25 - 3 MB)
        wf = {}
        ht = {}

        def load_wf(b):
            c0, n = CBLK[b]
            t = wp.tile([128, n * 4 * W], bf16, tag=f"wf{b}")
            nc.sync.dma_start(t[:], d_wf[b])
            wf[b] = t

        def load_ht(i):
            g0, n = HBLK[i]
            t = hp.tile([128, n * 512], bf16, tag=f"ht{i}")
            nc.sync.dma_start(t[:], d_ht[i])
            for g in range(g0, g0 + n):
                ht[g] = (t, (g - g0) * 512)

        load_wf(0)
        for i in range(len(HBLK)):
            load_ht(i)
        for b in range(1, len(CBLK)):
            load_wf(b)

        rr = [0]

        def copy(dst, src):
            if rr[0] % 2 == 0:
                nc.scalar.copy(dst, src)
            else:
                nc.vector.tensor_copy(dst, src)
            rr[0] += 1

        def mm_group(b, ci, g, pj, w):
            """4 accumulating matmuls: chunk (CBLK[b][0]+ci) x tile g"""
            htt, hoff = ht[g]
            for k in range(4):
                nc.tensor.matmul(
                    pj[:, 0:w],
                    htt[:, hoff + 128 * k:hoff + 128 * (k + 1)],
                    wf[b][:, (ci * 4 + k) * W:(ci * 4 + k) * W + w],
                    start=(k == 0), stop=(k == 3))

        # sweep 0: single chunk; results for GB0 token tiles batched into
        # one wide stage tile -> one large store per batch
        for gb in range(NG // GB0):
            st = s0.tile([128, GB0 * W], bf16, tag=f"st0_{gb}")
            for gi in range(GB0):
                g = gb * GB0 + gi
                pj = pp.tile([128, 512], f32, tag="pj")
                mm_group(0, 0, g, pj, W)
                copy(st[:, gi * W:(gi + 1) * W], pj[:, 0:W])
            nc.sync.dma_start(d_ob[0][gb], st[:])

        # blocked sweeps: n chunks per g, one large store per (b, g);
        # in the final block the last two g store per-chunk so the tail
        # drains with the matmuls
        bn = len(CBLK) - 1
        for b in range(1, len(CBLK)):
            c0, n = CBLK[b]
            ws = [CW[c] for c in range(c0, c0 + n)]
            offs = [sum(ws[:i]) for i in range(n + 1)]
            for g in range(NG):
                split = (b == bn and g >= NG - 2)
                st = sp[b].tile([128, offs[n]], bf16, tag=f"st{b}")
                for ci in range(n):
                    pj = pp.tile([128, 512], f32, tag="pj")
                    mm_group(b, ci, g, pj, ws[ci])
                    copy(st[:, offs[ci]:offs[ci + 1]], pj[:, 0:ws[ci]])
                    if split:
                        nc.sync.dma_start(
                            d_ob[b][g][:, offs[ci]:offs[ci + 1]],
                            st[:, offs[ci]:offs[ci + 1]])
                if not split:
                    nc.sync.dma_start(d_ob[b][g], st[:])

        for p in (pp, wu, sp[3], sp[2], sp[1], s0, hp, wp):
            p.release()

    nc.compile()
    return nc


def _host_scan(sequence, enc_h, enc_c, emb, W_ih0, W_hh0, b_ih0, b_hh0,
               W_ih1, W_hh1, b_ih1, b_hh1):
    """Mirror of the reference LSTM scan, float32 numpy. Returns
    h1 outputs [S, B, H]."""
    f32 = np.float32
    seq = np.asarray(sequence)
    emb = np.asarray(emb, f32)
    Wih0 = np.asarray(W_ih0, f32).T     # [E+H, 4H]
    Whh0 = np.asarray(W_hh0, f32).T     # [H, 4H]
    Wih1 = np.asarray(W_ih1, f32).T
    Whh1 = np.asarray(W_hh1, f32).T
    b0 = np.asarray(b_ih0, f32) + np.asarray(b_hh0, f32)
    b1 = np.asarray(b_ih1, f32) + np.asarray(b_hh1, f32)

    def sig(x):
        return 1.0 / (1.0 + np.exp(-x))

    def cell(g, c):
        i, f, gg, o = np.split(g, 4, axis=-1)
        c2 = sig(f) * c + sig(i) * np.tanh(gg)
        h2 = sig(o) * np.tanh(c2)
        return h2, c2

    h0 = np.asarray(enc_h[0], f32).copy()
    h1 = np.asarray(enc_h[1], f32).copy()
    c0 = np.asarray(enc_c[0], f32).copy()
    c1 = np.asarray(enc_c[1], f32).copy()
    feed = np.zeros((B, H), f32)

    x = emb[seq]                        # [B, S, E]
    outs = np.empty((S, B, H), f32)
    for t in range(S):
        inp = np.concatenate([x[:, t, :], feed], axis=1)       # [B, E+H]
        g0 = inp @ Wih0 + h0 @ Whh0 + b0
        h0, c0 = cell(g0, c0)
        g1 = h0 @ Wih1 + h1 @ Whh1 + b1
        h1, c1 = cell(g1, c1)
        feed = h1
        outs[t] = h1
    return outs


def _host_prep(outs, Wf):
    bfl = ml_dtypes.bfloat16
    # hT[g, h, 128k+j] = outs[4g+s, b, 128k+h] with j = 32s+b
    hT = np.ascontiguousarray(
        outs.reshape(NG, 4, B, 4, 128).transpose(0, 4, 3, 1, 2)
        .reshape(NG, 128, 512)).astype(bfl)
    htb = {}
    for i, (g0, n) in enumerate(HBLK):
        htb[f"hTb{i}"] = np.ascontiguousarray(
            hT[g0:g0 + n].transpose(1, 0, 2).reshape(128, n * 512))

    Wfp = np.zeros((VPAD, H), np.float32)
    Wfp[:V] = np.asarray(Wf, np.float32)

    in_maps = []
    for cidx in range(NC_):
        shard = Wfp[cidx * VSH:(cidx + 1) * VSH]      # [VSH, H]
        # pad shard columns to the 13 x 484 slot grid
        sT = np.zeros((H, NCH * W), np.float32)
        sT[:, :VSH] = shard.T
        kpcw = sT.reshape(4, 128, NCH, W)             # [k, p, c, w]
        m = dict(htb)
        for b, (c0, n) in enumerate(CBLK):
            m[f"wfb{b}"] = np.ascontiguousarray(
                kpcw[:, :, c0:c0 + n, :].transpose(1, 2, 0, 3)
                .reshape(128, n * 4 * W)).astype(bfl)
        in_maps.append(m)
    return in_maps


last_results = None


def kernel(**inputs):
    from concourse.bass_utils import run_bass_kernel_spmd

    if "nc" not in _cache:
        _cache["nc"] = _build_program()
    nc = _cache["nc"]

    outs = _host_scan(
        inputs["sequence"], inputs["enc_h"], inputs["enc_c"], inputs["emb"],
        inputs["W_ih0"], inputs["W_hh0"], inputs["b_ih0"], inputs["b_hh0"],
        inputs["W_ih1"], inputs["W_hh1"], inputs["b_ih1"], inputs["b_hh1"])
    in_maps = _host_prep(outs, inputs["Wf"])

    trace = bool(int(os.environ.get("K_TRACE", "0")))
    res = run_bass_kernel_spmd(nc, in_maps, core_ids=list(range(NC_)),
                               trace=trace)
    global last_results
    last_results = res

    # assemble: per core, blocks -> [NG, 128, 6283]; token tile g has
    # tokens (4g+s)*32+b at j=32s+b, so (g, j) flattens to (S, B) order.
    shards = []
    for cidx in range(NC_):
        parts = [res.results[cidx]["ob0"]
                 .reshape(NG // GB0, 128, GB0, W)
                 .transpose(0, 2, 1, 3).reshape(NG, 128, W)]
        parts += [res.results[cidx][f"ob{b}"]
                  for b in range(1, len(CBLK))]
        lt = np.concatenate(parts, axis=2)             # [NG, 128, 6283]
        sbv = lt.reshape(S, B, VSH)
        shards.append(sbv.transpose(1, 0, 2))          # -> [B, S, VSH]
    full = np.concatenate(shards, axis=2)[:, :, :V].astype(np.float32)
    bfv = np.asarray(inputs["bf"], np.float32)
    if np.any(bfv):
        full = full + bfv[None, None, :]
    return np.ascontiguousarray(full)
